# revision 3
# baseline (speedup 1.0000x reference)
"""Trainium2 Bass kernel for nn_DemoPredictor (GRU + negative-sampling loss).

Strategy: data-parallel over batch B=512 across 8 NeuronCores (64 rows each).
Per core:
  - embedding rows gathered from the (replicated) item_emb table via
    indirect DMA, 128 rows (= 2 timesteps x 64 batch) per transfer
  - gathered rows PE-transposed to E-major, input projection
    gi = embed @ w_ih.T + biases computed per 2-step pair into PSUM
    (fp32r matmuls), copied to SBUF
  - sequential GRU over T=200 steps: gh = h @ w_hh.T as fp32r matmuls
    streaming the weights (stationary hT), gates on ACT/DVE, h'
    PE-transposed back to K-major for the next step
  - last-valid-step h accumulated via one-hot weights, tiny output head
    + log-sigmoid loss reduction on device; per-core [sum_contrib,
    sum_valid] returned and combined on host.
"""
import numpy as np

import concourse.bass as bass
import concourse.mybir as mybir
from concourse import bacc, tile
from concourse.masks import make_identity
from concourse.bass_utils import run_bass_kernel_spmd

F32 = mybir.dt.float32
F32R = mybir.dt.float32r
I32 = mybir.dt.int32
AF = mybir.ActivationFunctionType
OP = mybir.AluOpType

# Problem shapes (hardcoded per spec)
B, T = 512, 200
E, H = 256, 512
VOCAB = 100000
L, NN = 20, 4
NCORES = 8
BL = B // NCORES          # 64 rows per core
PAIRS = T // 2            # 100
H3 = 3 * H                # 1536
EC = E // 128             # 2 contraction chunks for E
HC = H // 128             # 4 contraction chunks for H


def build_nc():
    nc = bacc.Bacc(
        "TRN2", target_bir_lowering=False, debug=False, enable_asserts=False
    )
    # DRAM inputs (per core)
    emb = nc.dram_tensor("emb", [VOCAB, E], F32, kind="ExternalInput")
    xidx = nc.dram_tensor("xidx", [128, PAIRS], I32, kind="ExternalInput")
    wihT = nc.dram_tensor("wihT", [E, H3], F32, kind="ExternalInput")
    whhT = nc.dram_tensor("whhT", [H, H3], F32, kind="ExternalInput")
    # biases: [brz (1024) | bin (512) | bhn (512)]
    biases = nc.dram_tensor("biases", [1, 2048], F32, kind="ExternalInput")
    woutT = nc.dram_tensor("woutT", [H, L], F32, kind="ExternalInput")
    sel = nc.dram_tensor("sel", [BL, T], F32, kind="ExternalInput")
    yv = nc.dram_tensor("yv", [BL, L], F32, kind="ExternalInput")
    ob = nc.dram_tensor("ob", [BL, L], F32, kind="ExternalInput")
    negs = nc.dram_tensor("negs", [BL, NN * L], F32, kind="ExternalInput")
    out = nc.dram_tensor("out", [2, 1], F32, kind="ExternalOutput")

    with tile.TileContext(nc) as tc:
        with (
            tc.tile_pool(name="const", bufs=1) as cp,
            tc.tile_pool(name="raw", bufs=3) as rawp,
            tc.tile_pool(name="embt", bufs=2) as embtp,
            tc.tile_pool(name="gis", bufs=2) as gisp,
            tc.tile_pool(name="state", bufs=2) as stp,
            tc.tile_pool(name="gates", bufs=2) as gp,
            tc.tile_pool(name="fin", bufs=1) as fp_,
            tc.tile_pool(name="ps_gi", bufs=2, space="PSUM") as ps_gi,
            tc.tile_pool(name="ps_embt", bufs=1, space="PSUM") as ps_embt,
            tc.tile_pool(name="ps_rec", bufs=1, space="PSUM") as ps_rec,
            tc.tile_pool(name="ps_ht", bufs=2, space="PSUM") as ps_ht,
        ):
            # ---- constants / one-time loads ----
            ident = cp.tile([128, 128], F32)
            make_identity(nc, ident[:])
            idx_sb = cp.tile([128, PAIRS], I32)
            nc.sync.dma_start(out=idx_sb[:], in_=xidx[:])
            wih_sb = cp.tile([128, EC * H3], F32R)
            nc.gpsimd.dma_start(
                out=wih_sb[:].rearrange("p (c n) -> p c n", c=EC),
                in_=wihT[:].rearrange("(c p) n -> p c n", p=128),
            )
            whh_sb = cp.tile([128, HC * H3], F32R)
            nc.gpsimd.dma_start(
                out=whh_sb[:].rearrange("p (c n) -> p c n", c=HC),
                in_=whhT[:].rearrange("(c p) n -> p c n", p=128),
            )
            bias_sb = cp.tile([1, 2048], F32R)
            nc.gpsimd.dma_start(out=bias_sb[:], in_=biases[:])
            ones_f = cp.tile([1, 128], F32)
            nc.vector.memset(ones_f[:], 1.0)
            ones1 = cp.tile([1, 128], F32R)
            nc.vector.tensor_copy(ones1[:], ones_f[:])
            sel_sb = cp.tile([BL, T], F32)
            nc.sync.dma_start(out=sel_sb[:], in_=sel[:])
            y_sb = cp.tile([BL, L], F32)
            nc.sync.dma_start(out=y_sb[:], in_=yv[:])
            ob_sb = cp.tile([BL, L], F32)
            nc.sync.dma_start(out=ob_sb[:], in_=ob[:])
            negs_sb = cp.tile([BL, NN * L], F32)
            nc.sync.dma_start(out=negs_sb[:], in_=negs[:])
            wout_sb = cp.tile([128, HC * L], F32)
            nc.sync.dma_start(
                out=wout_sb[:].rearrange("p (c n) -> p c n", c=HC),
                in_=woutT[:].rearrange("(c p) n -> p c n", p=128),
            )
            h0 = cp.tile([BL, H], F32)
            nc.vector.memset(h0[:], 0.0)
            acc = cp.tile([BL, H], F32)
            nc.vector.memset(acc[:], 0.0)

            h_cur = h0
            hT_cur = None

            def gi_pair(p):
                """Gather + transpose + input projection for pair p.
                Returns dict gate -> SBUF tile [128, 512] (rows 0:64 =
                step 2p, rows 64:128 = step 2p+1), incl. biases."""
                raw = rawp.tile([128, E], F32, tag="raw")
                nc.gpsimd.indirect_dma_start(
                    out=raw[:],
                    out_offset=None,
                    in_=emb[:],
                    in_offset=bass.IndirectOffsetOnAxis(
                        ap=idx_sb[:, p:p + 1], axis=0
                    ),
                )
                embt = embtp.tile([128, EC * 128], F32R, tag="embt")
                for c in range(EC):
                    tp = ps_embt.tile([128, 128], F32, space="PSUM", tag="te")
                    nc.tensor.transpose(
                        tp[:], raw[:, c * 128:(c + 1) * 128], ident[:]
                    )
                    nc.vector.tensor_copy(
                        embt[:, c * 128:(c + 1) * 128], tp[:]
                    )
                gi = {}
                # bias slices within bias_sb: r=[0:512], z=[512:1024],
                # n=[1024:1536] (b_ih part; b_hh n-part is at [1536:2048])
                for g, bofs in (("r", 0), ("z", 512), ("n", 1024)):
                    bank = ps_gi.tile([128, 512], F32, space="PSUM", tag="gi")
                    nc.tensor.matmul(
                        bank[:], ones1[:], bias_sb[:, bofs:bofs + 512],
                        start=True, stop=False,
                    )
                    for c in range(EC):
                        nc.tensor.matmul(
                            bank[:],
                            embt[:, c * 128:(c + 1) * 128],
                            wih_sb[:, c * H3 + (bofs):c * H3 + bofs + 512],
                            start=False, stop=(c == EC - 1),
                        )
                    sb_e = gisp.tile([BL, 512], F32, tag=f"gi{g}e")
                    nc.vector.tensor_copy(sb_e[:], bank[0:BL, :])
                    sb_o = gisp.tile([BL, 512], F32, tag=f"gi{g}o")
                    nc.vector.tensor_copy(sb_o[:], bank[BL:128, :])
                    gi[g] = (sb_e, sb_o)
                return gi

            gi_cur = gi_pair(0)
            gi_next = None

            for t in range(T):
                p, half = divmod(t, 2)
                gir = gi_cur["r"][half][:]
                giz = gi_cur["z"][half][:]
                gin = gi_cur["n"][half][:]

                # recurrence matmuls (skip at t=0 where h=0)
                if t > 0:
                    rb = ps_rec.tile([BL, 512], F32, space="PSUM", tag="rb")
                    for c in range(HC):
                        nc.tensor.matmul(
                            rb[:],
                            hT_cur[:, c * BL:(c + 1) * BL],
                            whh_sb[:, c * H3:c * H3 + 512],
                            start=(c == 0), stop=(c == HC - 1),
                        )
                    zb = ps_rec.tile([BL, 512], F32, space="PSUM", tag="zb")
                    for c in range(HC):
                        nc.tensor.matmul(
                            zb[:],
                            hT_cur[:, c * BL:(c + 1) * BL],
                            whh_sb[:, c * H3 + 512:c * H3 + 1024],
                            start=(c == 0), stop=(c == HC - 1),
                        )
                # ghn bank always (bias b_hn even at t=0)
                gb = ps_rec.tile([BL, 512], F32, space="PSUM", tag="gb")
                nc.tensor.matmul(
                    gb[:], ones1[:, :BL], bias_sb[:, 1536:2048],
                    start=True, stop=(t == 0),
                )
                if t > 0:
                    for c in range(HC):
                        nc.tensor.matmul(
                            gb[:],
                            hT_cur[:, c * BL:(c + 1) * BL],
                            whh_sb[:, c * H3 + 1024:c * H3 + 1536],
                            start=False, stop=(c == HC - 1),
                        )

                # gates
                r_t = gp.tile([BL, H], F32, tag="r")
                s_t = gp.tile([BL, H], F32, tag="s")  # s = 1 - z
                if t > 0:
                    rpre = gp.tile([BL, H], F32, tag="rpre")
                    nc.vector.tensor_add(rpre[:], gir, rb[:])
                    nc.scalar.activation(r_t[:], rpre[:], AF.Sigmoid)
                    zpre = gp.tile([BL, H], F32, tag="zpre")
                    nc.vector.tensor_add(zpre[:], giz, zb[:])
                    nc.scalar.activation(s_t[:], zpre[:], AF.Sigmoid, scale=-1.0)
                else:
                    nc.scalar.activation(r_t[:], gir, AF.Sigmoid)
                    nc.scalar.activation(s_t[:], giz, AF.Sigmoid, scale=-1.0)
                # w = (s - 1) * h   (= -(1-s)h)
                w_t = gp.tile([BL, H], F32, tag="w")
                nc.vector.scalar_tensor_tensor(
                    out=w_t[:], in0=s_t[:], scalar=1.0, in1=h_cur[:],
                    op0=OP.subtract, op1=OP.mult,
                )
                # u = r * ghn ; v = u + gin ; n = tanh(v)
                u_t = gp.tile([BL, H], F32, tag="u")
                nc.vector.tensor_mul(u_t[:], r_t[:], gb[:])
                v_t = gp.tile([BL, H], F32, tag="v")
                nc.vector.tensor_add(v_t[:], u_t[:], gin)
                # tanh(v) = 2*sigmoid(2v) - 1 (keeps ACT in one table)
                n2_t = gp.tile([BL, H], F32, tag="n2")
                nc.scalar.activation(n2_t[:], v_t[:], AF.Sigmoid, scale=2.0)
                n_t = gp.tile([BL, H], F32, tag="n")
                nc.vector.tensor_scalar(
                    out=n_t[:], in0=n2_t[:], scalar1=2.0, scalar2=-1.0,
                    op0=OP.mult, op1=OP.add,
                )
                # h' = s*n - w
                c_t = gp.tile([BL, H], F32, tag="c")
                nc.vector.tensor_mul(c_t[:], s_t[:], n_t[:])
                h_new = stp.tile([BL, H], F32, tag="h")
                nc.vector.tensor_sub(h_new[:], c_t[:], w_t[:])
                # acc += sel[:, t] * h'
                nc.vector.scalar_tensor_tensor(
                    out=acc[:], in0=h_new[:], scalar=sel_sb[:, t:t + 1],
                    in1=acc[:], op0=OP.mult, op1=OP.add,
                )
                # hT for next step
                if t < T - 1:
                    htp = ps_ht.tile([128, HC * BL], F32, space="PSUM",
                                     tag="ht")
                    for c in range(HC):
                        nc.tensor.transpose(
                            htp[:, c * BL:(c + 1) * BL],
                            h_new[:, c * 128:(c + 1) * 128],
                            ident[:BL, :BL],
                        )
                    hT_new = stp.tile([128, HC * BL], F32R, tag="hT")
                    nc.vector.tensor_copy(hT_new[:], htp[:])
                    hT_cur = hT_new
                h_cur = h_new

                # prefetch input projection for the next pair
                if half == 0 and p + 1 < PAIRS:
                    gi_next = gi_pair(p + 1)
                elif half == 1:
                    gi_cur = gi_next

            # ---- epilogue: head + loss ----
            # accT via PE transposes (fp32)
            accT_ps = ps_ht.tile([128, HC * BL], F32, space="PSUM", tag="ht")
            for c in range(HC):
                nc.tensor.transpose(
                    accT_ps[:, c * BL:(c + 1) * BL],
                    acc[:, c * 128:(c + 1) * 128],
                    ident[:BL, :BL],
                )
            accT = fp_.tile([128, HC * BL], F32)
            nc.vector.tensor_copy(accT[:], accT_ps[:])
            wu_ps = ps_rec.tile([BL, L], F32, space="PSUM", tag="rb")
            for c in range(HC):
                nc.tensor.matmul(
                    wu_ps[:],
                    accT[:, c * BL:(c + 1) * BL],
                    wout_sb[:, c * L:(c + 1) * L],
                    start=(c == 0), stop=(c == HC - 1),
                )
            wc = fp_.tile([BL, L], F32)
            nc.vector.tensor_mul(wc[:], wu_ps[:], ob_sb[:])
            # pos: sum_j softplus(-wc*y)
            # softplus(x) = ln(1 + exp(x)); args are O(1) so no overflow
            py = fp_.tile([BL, L], F32)
            nc.vector.tensor_mul(py[:], wc[:], y_sb[:])
            e1 = fp_.tile([BL, L], F32)
            nc.scalar.activation(e1[:], py[:], AF.Exp, scale=-1.0)
            e1p = fp_.tile([BL, L], F32)
            nc.vector.tensor_scalar_add(e1p[:], e1[:], 1.0)
            lg1 = fp_.tile([BL, L], F32)
            pos_sum = fp_.tile([BL, 1], F32)
            nc.scalar.activation(
                lg1[:], e1p[:], AF.Ln, accum_out=pos_sum[:]
            )
            # neg: sum_{k,j} softplus(negs*wc)
            m_t = fp_.tile([BL, NN * L], F32)
            for k in range(NN):
                nc.vector.tensor_mul(
                    m_t[:, k * L:(k + 1) * L],
                    negs_sb[:, k * L:(k + 1) * L],
                    wc[:],
                )
            e2 = fp_.tile([BL, NN * L], F32)
            nc.scalar.activation(e2[:], m_t[:], AF.Exp)
            e2p = fp_.tile([BL, NN * L], F32)
            nc.vector.tensor_scalar_add(e2p[:], e2[:], 1.0)
            lg2 = fp_.tile([BL, NN * L], F32)
            neg_sum = fp_.tile([BL, 1], F32)
            nc.scalar.activation(
                lg2[:], e2p[:], AF.Ln, accum_out=neg_sum[:]
            )
            # valid = sign(|sum_j wc|)
            vs = fp_.tile([BL, 1], F32)
            nc.vector.tensor_reduce(
                vs[:], wc[:], axis=mybir.AxisListType.X, op=OP.add
            )
            va = fp_.tile([BL, 1], F32)
            nc.scalar.activation(va[:], vs[:], AF.Abs)
            valid = fp_.tile([BL, 1], F32)
            nc.scalar.activation(valid[:], va[:], AF.Sign)
            # contrib = (pos_sum + neg_sum) * valid  [= -(pos_log+neg_log)*valid]
            tot = fp_.tile([BL, 1], F32)
            nc.vector.tensor_add(tot[:], pos_sum[:], neg_sum[:])
            red_in = fp_.tile([BL, 2], F32)
            nc.vector.tensor_mul(red_in[:, 0:1], tot[:], valid[:])
            nc.vector.tensor_copy(red_in[:, 1:2], valid[:])
            onesB = fp_.tile([BL, 1], F32)
            nc.vector.memset(onesB[:], 1.0)
            red_ps = ps_rec.tile([2, 1], F32, space="PSUM", tag="zb")
            nc.tensor.matmul(
                red_ps[:], red_in[:], onesB[:], start=True, stop=True
            )
            red_sb = fp_.tile([2, 1], F32)
            nc.vector.tensor_copy(red_sb[:], red_ps[:])
            nc.sync.dma_start(out=out[:], in_=red_sb[:])
    nc.finalize()
    return nc


_NC_CACHE = None


def _get_nc():
    global _NC_CACHE
    if _NC_CACHE is None:
        _NC_CACHE = build_nc()
    return _NC_CACHE


def make_in_maps(x, x_mask, y, ob, neg_samples, item_emb,
                 w_ih, w_hh, b_ih, b_hh, w_out):
    """Host-side sharding / layout prep. Returns list of 8 input dicts."""
    item_emb = np.ascontiguousarray(np.asarray(item_emb, dtype=np.float32))
    wihT = np.ascontiguousarray(np.asarray(w_ih, np.float32).T)   # [E, 3H]
    whhT = np.ascontiguousarray(np.asarray(w_hh, np.float32).T)   # [H, 3H]
    b_ih = np.asarray(b_ih, np.float32)
    b_hh = np.asarray(b_hh, np.float32)
    brz = b_ih[:2 * H] + b_hh[:2 * H]
    biases = np.concatenate(
        [brz, b_ih[2 * H:], b_hh[2 * H:]]
    ).reshape(1, 2048).astype(np.float32)
    woutT = np.ascontiguousarray(np.asarray(w_out, np.float32).T)  # [H, L]
    x = np.asarray(x)
    x_len = np.asarray(x_mask).astype(np.int64).sum(axis=1)        # [B]
    last = np.clip(x_len - 1, 0, T - 1)
    sel_full = np.zeros((B, T), np.float32)
    sel_full[np.arange(B), last] = 1.0
    y = np.asarray(y, np.float32)
    ob = np.asarray(ob, np.float32)
    negs = np.asarray(neg_samples, np.float32).reshape(B, NN * L)

    in_maps = []
    for c in range(NCORES):
        rs = slice(c * BL, (c + 1) * BL)
        xl = x[rs].astype(np.int32)                     # [BL, T]
        # xidx[row, p]: rows 0:64 -> t=2p, rows 64:128 -> t=2p+1
        xt = xl.T.reshape(PAIRS, 2, BL)                 # [p, half, b]
        xidx = np.ascontiguousarray(
            xt.transpose(1, 2, 0).reshape(128, PAIRS)
        )
        in_maps.append({
            "emb": item_emb,
            "xidx": xidx,
            "wihT": wihT,
            "whhT": whhT,
            "biases": biases,
            "woutT": woutT,
            "sel": np.ascontiguousarray(sel_full[rs]),
            "yv": np.ascontiguousarray(y[rs]),
            "ob": np.ascontiguousarray(ob[rs]),
            "negs": np.ascontiguousarray(negs[rs]),
        })
    return in_maps


def combine_outputs(results):
    num = sum(float(r["out"][0, 0]) for r in results)
    nva = sum(float(r["out"][1, 0]) for r in results)
    return np.float32(num / max(nva, 1.0))


def kernel(**inputs) -> np.ndarray:
    nc = _get_nc()
    in_maps = make_in_maps(**inputs)
    res = run_bass_kernel_spmd(nc, in_maps, core_ids=list(range(NCORES)))
    return np.asarray(combine_outputs(res.results))
